# revision 25
# baseline (speedup 1.0000x reference)
"""Trainium2 Bass kernel for nn_BlockLTN (gnn_message_passing).

Math:
    z[o,v,c] = sum_{k,d} x[v,k,d] * W[o,d,k,c] + sum_d b[o,c,d]
    out[e,c,o] = sum_v G[e,v] * z[o,v,c]

Folded:  out[e, c*8+o] = G[e,:] @ Z2[:, c*8+o]
  where  Z2[v, c*8+o] = (x.reshape(V,KD) @ W.transpose(2,1,3,0).reshape(KD,CO))[v, c*8+o]
                        + b.sum(-1).T.reshape(CO)[c*8+o]

The dominant work is the [E,V] @ [V, CO] GEMM over the 256 MB boundary
operator G (68.7 GFLOP); Z2 is a 4.3 GFLOP preprocessing folded on host.
Sharding (per hint): G and out row-wise over E across 8 cores (data
parallel over out-simplices); Z2 (8 MB bf16) replicated; no collectives.
G ships as bf16 lhsT (host transpose+cast) so the TensorE runs at its
78.6 TF/s bf16 peak; accumulation is fp32 in PSUM (rel err ~2.3e-3).
fp8 was evaluated and rejected: e4m3 quantization of G/Z2 gives ~3.7%
rel err (gate is 2e-2) and the error-compensated 3-term variant costs
1.5x the bf16 matmul stream.

SYNCHRONIZATION (the part that was subtly wrong before): a DMA's
`.then_inc(sem, 16)` is performed as SIXTEEN independent +1 increments,
one from each SDMA engine as it finishes its own 8-partition share.
With several DMAs in flight on one semaphore, increments from a later
DMA can satisfy an earlier DMA's cumulative threshold while one lagging
SDMA engine still owes its 8 partitions -> the consumer reads 8 stale
partition rows (observed as rank-8 corruption under NTFF profiling,
which skews SDMA engine progress). Therefore every DMA that a consumer
waits on gets its OWN single-use semaphore (threshold 16 == fully
landed); only the final all-DMAs-done wait (s_out) uses a cumulative
count, which is sound because the final total requires every increment.

Device kernel (per core, E_loc = 1024 rows), hand-scheduled nc.Block:
  - GT and Z2 are shipped partition-major ([128, 64, cols]) so every
    descriptor moves 2-8 KB contiguous per partition.
  - Z2 [128, 64, 512] bf16 resident in SBUF on the scalar HWDGE queue;
    groups past the ramp are paced by PE chunk consumption (s_mm) to
    keep the early DMA burst from starving the PE's weight prefetch
    (unpaced, the first ~12 matmuls ran at 2x time).
  - GT chunks [128, 1024] bf16 stream through a 16-slot ring on the sync
    HWDGE queue (singles/pairs through the ramp, then quads).
  - 512 bf16 matmuls accumulate into all 8 PSUM banks (216 ns/mm =
    silicon limit for N=512).
  - Tail: VectorE copies PSUM tiles 0-3 to SBUF fp16 (SyncE ships them
    in pairs), ScalarE copies+ships tiles 4-7. Out is fp16; the host
    upcasts to f32 (adds ~0.05% error against a 2e-2 gate).

Timing on the 8-core axon TRN2 (fast clock mode): ~130.5-133 us HW
exec = ~11 us preamble (framework all-engine barrier waits ~6 us for
the GpSimd Q7 boot, then first-chunk DMA + completion receipt) +
~113 us matmul stream (+0.5 us residual ramp/stall) + ~7 us tail
(last PSUM evac + out DMA receipt + NEFF completion detection). Some
processes land in a ~20% slower DVFS mode (~155 us) regardless of
kernel content. Two further experiments were tried and REVERTED:
issuing the first DMAs before the framework preamble barrier (races
the runtime's exec-start DMA init on the first execution -> scattered
garbage), and a reordered v=63 tail with minimal copy thresholds
(intermittently shipped stale tail columns of e-tile 7).
"""

import numpy as np
import ml_dtypes

V = 8192
E = 8192
K = 64
C = 64
D = 8
O = 8
KD = K * D    # 512
CO = C * O    # 512
N_CORES = 8
EL = E // N_CORES  # 1024 out-rows per core
N_VCHUNK = V // 128  # 64
N_ETILE = EL // 128  # 8

BF16 = ml_dtypes.bfloat16

_cache = {}

# GT dma plan: chunks 0 and 1 alone (so the PE can start as soon as 256 KB
# lands), pairs through the ramp, then quads (8 KB contiguous per
# partition per descriptor). The first N_GT_PRE dmas are issued before
# the preamble barrier.
GT_DMAS = [(0, 1), (1, 1), (2, 2), (4, 2), (6, 2)] + [
    (8 + 4 * t, 4) for t in range((N_VCHUNK - 8) // 4)
]
assert sum(n for _, n in GT_DMAS) == N_VCHUNK
N_GT_PRE = 3   # chunks 0-3
_GT_IDX = {}
for _d, (_a, _n) in enumerate(GT_DMAS):
    for _c in range(_a, _a + _n):
        _GT_IDX[_c] = _d

# z2 load groups: small first so the v=0 matmuls start ASAP and the
# ramp chunks (4-7) are not stuck behind a 1 MB transfer
Z2_GROUPS = [1, 1, 2, 2, 2] + [4] * 14
assert sum(Z2_GROUPS) == N_VCHUNK
N_Z2_PRE = 3   # chunks 0-3

# et emission order for the final (v=63) chunk. A reordered tail
# ([4,5,6,7,0,1,2,3] with minimal copy thresholds) shaved ~1 us but
# intermittently shipped garbage tail columns of e-tile 7 on first
# executions (copy/DMA racing data visibility while the PE is still
# active); the plain order with each copy gated on the FULL v=63 set
# finishing has soaked clean.
FIN_ORDER = list(range(N_ETILE))
FIN_THRESH = {et: et + 1 for et in FIN_ORDER}


def _gt_dma_idx(v):
    """Index of the GT dma that carries chunk v."""
    return _GT_IDX[v]


def _build_bass_raw():
    import os
    import concourse.mybir as mybir
    from concourse import bacc

    f32 = mybir.dt.float32
    fp16 = mybir.dt.float16
    bf16 = mybir.dt.bfloat16

    nc = bacc.Bacc("TRN2", target_bir_lowering=False)

    # partition-major layouts prepared on host
    gt = nc.dram_tensor("gt", (128, N_VCHUNK, EL), bf16, kind="ExternalInput")
    z2 = nc.dram_tensor("z2", (128, N_VCHUNK, CO), bf16, kind="ExternalInput")
    out = nc.dram_tensor("out", (128, N_ETILE, CO), fp16, kind="ExternalOutput")

    NSLOT = 16  # gt ring depth (chunks of [128, EL] bf16, 2KB/partition each)
    z2sb = nc.alloc_sbuf_tensor("z2sb", [128, N_VCHUNK, CO], bf16)
    gtsb = nc.alloc_sbuf_tensor("gtsb", [128, NSLOT, EL], bf16)
    osb = nc.alloc_sbuf_tensor("osb", [128, N_ETILE, CO], fp16)
    ps = [nc.alloc_psum_tensor(f"ps{i}", [128, CO], f32) for i in range(N_ETILE)]

    # single-use DMA-completion sems (see module docstring)
    gts = [nc.alloc_semaphore(f"s_gt{d}") for d in range(len(GT_DMAS))]
    # chunk 0 ships as two column halves so the PE can start on e-tiles
    # 0-3 after only 128 KB lands; gts[0] covers cols 0-511, s_g0b the rest
    s_g0b = nc.alloc_semaphore("s_g0b")
    z2s = [nc.alloc_semaphore(f"s_z2{g}") for g in range(len(Z2_GROUPS))]
    s_mm = nc.alloc_semaphore("s_mm")    # PE consumed chunk v: v+1
    s_fin = nc.alloc_semaphore("s_fin")  # v=63 matmuls retired, FIN_ORDER
    s_cpv = nc.alloc_semaphore("s_cpv")  # DVE psum->sbuf copies done
    s_out = nc.alloc_semaphore("s_out")  # out DMAs landed: 16 each

    all_sems = gts + [s_g0b] + z2s + [s_mm, s_fin, s_cpv, s_out]
    nums = [s.num for s in all_sems]
    assert nums == list(range(nums[0], nums[0] + len(nums))), nums
    sem_range = range(nums[0], nums[-1] + 1)
    # cleared at start: only sems without pre-barrier producers (the
    # gt/z2 sems rely on NEFF-load zeroing + the end-of-kernel clear)
    sem_range_start = range(s_mm.num, s_out.num + 1)

    groups = []
    v0 = 0
    for zg in Z2_GROUPS:
        groups.append((v0, zg))
        v0 += zg

    def gt_dma(eng, d):
        a, n = GT_DMAS[d]
        sl = a % NSLOT
        eng.dma_start(gtsb[:, sl:sl + n, :], gt[:, a:a + n, :]).then_inc(
            gts[d], 16
        )

    def z2_dma(eng, g):
        v0g, zg = groups[g]
        eng.dma_start(
            z2sb[:, v0g:v0g + zg, :], z2[:, v0g:v0g + zg, :]
        ).then_inc(z2s[g], 16)

    # Pre-barrier issue of the first dmas (saves ~6 us of preamble) is
    # DISABLED: descriptors issued in the first ~2.5 us of the first
    # execution race the runtime's own exec-start DMA initialization and
    # corrupt scattered partitions across all cores.
    use_pre = os.environ.get("KOPT_PRE", "0") != "0"
    n_gt_pre = N_GT_PRE if use_pre else 0
    n_z2_pre = N_Z2_PRE if use_pre else 0

    if use_pre:
        # Emit the first GT/Z2 dmas now (they land in the entry bb after
        # the framework preamble + barrier), then relocate them to just
        # BEFORE the preamble barrier so they stream during the ~6 us
        # GpSimd boot the barrier waits out.
        entry = nc.main_func.blocks[0]
        barrier_at = next(
            i for i, ins in enumerate(entry.instructions)
            if type(ins).__name__ == "InstDrain"
        )
        pre_n = len(entry.instructions)
        nc.sync.sem_clear(sem_range_start)
        for d in range(n_gt_pre):
            gt_dma(nc.sync, d)
        for g in range(n_z2_pre):
            z2_dma(nc.scalar, g)
        mine = entry.instructions[pre_n:]
        assert len(mine) == 1 + n_gt_pre + n_z2_pre, len(mine)
        del entry.instructions[pre_n:]
        for off, ins in enumerate(mine):
            entry.instructions.insert(barrier_at + off, ins)

    with nc.Block(name="k", no_gpsimd_drain=True) as blk:

        @blk.sync
        def _(eng):
            if not use_pre:
                eng.sem_clear(sem_range_start)
            for d in range(n_gt_pre, len(GT_DMAS)):
                a, n = GT_DMAS[d]
                if a >= NSLOT:
                    eng.wait_ge(s_mm, a + n - NSLOT)
                elif a >= 8:
                    # pace the ramp: chunks 8-15 are not needed for
                    # ~14 us; issuing them at exec start contributes to
                    # the DMA burst that starves the PE weight prefetch
                    # (first ~12 matmuls ran at 2x time)
                    eng.wait_ge(s_mm, a - 7)
                if d == 0:
                    eng.dma_start(
                        gtsb[:, 0, 0:512], gt[:, 0, 0:512]
                    ).then_inc(gts[0], 16)
                    eng.dma_start(
                        gtsb[:, 0, 512:EL], gt[:, 0, 512:EL]
                    ).then_inc(s_g0b, 16)
                else:
                    gt_dma(eng, d)
            for k, et in enumerate((0, 2)):
                eng.wait_ge(s_cpv, 2 * (k + 1))
                eng.dma_start(
                    out[:, et:et + 2, :], osb[:, et:et + 2, :]
                ).then_inc(s_out, 16)
            eng.wait_ge(s_out, 16 * 5)
            # leave sems zeroed so a re-execution of the loaded NEFF works
            eng.sem_clear(sem_range)

        @blk.scalar
        def _(eng):
            for g in range(n_z2_pre, len(groups)):
                c0 = groups[g][0]
                if 8 <= c0 < 12:
                    eng.wait_ge(s_mm, 1)
                elif c0 >= 12:
                    # pace Z2 groups by PE consumption: a group starting
                    # at chunk c0 issues ~10 consumed chunks (~17 us)
                    # before it is needed, keeping the early DMA burst
                    # from starving the PE's weight prefetch during the
                    # ramp (which ran the first ~12 matmuls at 2x time;
                    # pacing the ramp groups as well starved chunk 4)
                    eng.wait_ge(s_mm, c0 - 10)
                z2_dma(eng, g)
            for et in (4, 5, 6, 7):
                eng.wait_ge(s_fin, FIN_THRESH[et])
                eng.copy(osb[:, et, :], ps[et][:])
                if et == 5:
                    eng.dma_start(
                        out[:, 4:6, :], osb[:, 4:6, :]
                    ).then_inc(s_out, 16)
                elif et >= 6:
                    # tiles 6 and 7 ship as singles: the final,
                    # receipt-latency-bound dma is 128 KB instead of 256
                    eng.dma_start(
                        out[:, et:et + 1, :], osb[:, et:et + 1, :]
                    ).then_inc(s_out, 16)

        @blk.tensor
        def _(eng):
            landed = 0
            g = 0
            for v in range(N_VCHUNK):
                while v >= landed:
                    eng.wait_ge(z2s[g], 16)
                    landed += groups[g][1]
                    g += 1
                d = _gt_dma_idx(v)
                if v == GT_DMAS[d][0]:
                    # one wait per GT dma (chunks of the same dma share a
                    # completion sem; redundant waits cost PE issue time)
                    eng.wait_ge(gts[d], 16)
                last = v == N_VCHUNK - 1
                ets = FIN_ORDER if last else range(N_ETILE)
                for j, et in enumerate(ets):
                    if v == 0 and et == 4:
                        eng.wait_ge(s_g0b, 16)
                    mm = eng.matmul(
                        ps[et][:],
                        lhsT=gtsb[:, v % NSLOT, et * 128:(et + 1) * 128],
                        rhs=z2sb[:, v, :],
                        start=(v == 0),
                        stop=last,
                    )
                    if j == N_ETILE - 1 and not last:
                        mm.then_inc(s_mm, 1)
                    if last:
                        mm.then_inc(s_fin, 1)

        @blk.vector
        def _(eng):
            for et in (0, 1, 2, 3):
                eng.wait_ge(s_fin, FIN_THRESH[et])
                eng.tensor_copy(osb[:, et, :], ps[et][:]).then_inc(s_cpv, 1)

    nc.compile()
    return nc


def _prep_inputs(x, G, W, b):
    x = np.asarray(x, dtype=np.float32)
    G = np.asarray(G, dtype=np.float32)
    W = np.asarray(W, dtype=np.float32)
    b = np.asarray(b, dtype=np.float32)

    X2 = np.ascontiguousarray(x.reshape(V, KD))                 # [V, (k,d)]
    WM = np.ascontiguousarray(W.transpose(2, 1, 3, 0).reshape(KD, CO))  # [(k,d),(c,o)]
    bias = b.sum(axis=-1).T.reshape(CO)                          # [(c,o)]
    Z2 = (X2 @ WM + bias[None, :]).astype(BF16)                  # [V, CO]
    # partition-major: [128, n, CO], row v = n*128 + p
    Z2P = np.ascontiguousarray(Z2.reshape(N_VCHUNK, 128, CO).transpose(1, 0, 2))

    GT = G.T.astype(BF16)                                        # [V, E] contiguous
    in_maps = []
    for c in range(N_CORES):
        GTc = GT[:, c * EL:(c + 1) * EL]                          # [V, EL]
        GTP = np.ascontiguousarray(
            GTc.reshape(N_VCHUNK, 128, EL).transpose(1, 0, 2)    # [128, n, EL]
        )
        in_maps.append({"gt": GTP, "z2": Z2P})
    return in_maps


def _run(x, G, W, b, trace=False, trace_cores=None):
    from concourse.bass_utils import run_bass_kernel_spmd

    if "raw" not in _cache:
        _cache["raw"] = _build_bass_raw()
    nc = _cache["raw"]

    in_maps = _prep_inputs(x, G, W, b)
    kw = {}
    if trace_cores is not None:
        kw["trace_cores"] = trace_cores
    res = run_bass_kernel_spmd(
        nc, in_maps, core_ids=list(range(N_CORES)), trace=trace, **kw,
    )
    # out is [128, 8, 512] fp16 per core, row e_loc = et*128 + p
    outs = []
    for c in range(N_CORES):
        o = res.results[c]["out"]
        outs.append(np.ascontiguousarray(o.transpose(1, 0, 2)).reshape(EL, CO))
    out = np.concatenate(outs, axis=0).astype(np.float32)
    out = out.reshape(E, C, O)
    return out, res


def kernel(x, G, W, b):
    out, _ = _run(x, G, W, b, trace=False)
    return out
